# revision 6
# baseline (speedup 1.0000x reference)
"""Laplacian normalization kernel for Trainium2 (8 NeuronCores, SPMD).

out = D^-1/2 A D^-1/2 where D = diag(row sums of A), A: [8192, 8192] fp32.

Sharding: rows are split across 8 cores (1024 rows each). Each core:
  pass 1: stream its row block, reduce row sums (local degrees)
  AllGather the [1024] local degree chunks -> full [8192] degree vector
  compute isq = 1/sqrt(deg); broadcast isq across partitions for the
  column scale; row scale comes from the local degrees directly
  pass 2: out = (A * r[:, None]) * c[None, :] as one fused DVE op per stripe
"""

import sys

sys.path.insert(0, "/opt/trn_rl_repo")

import numpy as np

import concourse.bacc as bacc
import concourse.tile as tile
from concourse import mybir
from concourse.bass_utils import run_bass_kernel_spmd

N = 8192          # full matrix dim
CORES = 8
R = N // CORES    # rows per core: 1024
P = 128           # partitions
S = R // P        # stripes per core: 8
F32 = mybir.dt.float32

_CACHE = {}


def build_nc():
    if "nc" in _CACHE:
        return _CACHE["nc"]
    nc = bacc.Bacc(
        "TRN2", target_bir_lowering=False, debug=False, num_devices=CORES
    )
    a = nc.dram_tensor("a_block", [R, N], F32, kind="ExternalInput").ap()
    out = nc.dram_tensor("out_block", [R, N], F32, kind="ExternalOutput").ap()

    with tile.TileContext(nc) as tc:
        with (
            tc.tile_pool(name="dram", bufs=1, space="DRAM") as dram,
            tc.tile_pool(name="big", bufs=4) as big,
            tc.tile_pool(name="cpool", bufs=1) as cpool,
            tc.tile_pool(name="small", bufs=1) as small,
        ):
            deg_loc = dram.tile([R], F32)
            deg_all = dram.tile([N], F32, addr_space="Shared")
            isq_dram = dram.tile([N], F32)

            # ---- pass 1: row sums of each 128-row stripe ----
            # Big loads go on the nc.sync HWDGE ring; the tiny degree
            # writes go via SWDGE (gpsimd) so the FIFO ring of big loads
            # never stalls behind a reduce-dependent small DMA.
            dsb = small.tile([P, S], F32)  # local degrees, stripe s in col s
            for s in range(S):
                t = big.tile([P, N], F32, tag="stripe")
                nc.sync.dma_start(t[:], a[s * P : (s + 1) * P, :])
                nc.vector.reduce_sum(
                    out=dsb[:, s : s + 1], in_=t[:], axis=mybir.AxisListType.X
                )
                nc.gpsimd.dma_start(
                    deg_loc[s * P : (s + 1) * P].unsqueeze(1), dsb[:, s : s + 1]
                )

            # ---- collective: full degree vector ----
            nc.gpsimd.collective_compute(
                "AllGather",
                mybir.AluOpType.bypass,
                replica_groups=[list(range(CORES))],
                ins=[deg_loc[:].opt()],
                outs=[deg_all[:].opt()],
            )

            # ---- isq = 1/sqrt(deg_all), staged back through DRAM ----
            # All on SWDGE: independent of the big-load ring, so these
            # latency-critical steps run as soon as the collective lands.
            t64 = small.tile([P, N // P], F32)
            nc.gpsimd.dma_start(t64[:], deg_all[:].rearrange("(p f) -> p f", p=P))
            nc.vector.reciprocal(t64[:], t64[:])
            nc.scalar.sqrt(t64[:], t64[:])
            nc.gpsimd.dma_start(
                isq_dram[:].rearrange("(p f) -> p f", p=P), t64[:]
            )

            # column scale broadcast across all 128 partitions
            cb = cpool.tile([P, N], F32)
            nc.gpsimd.dma_start(
                cb[:], isq_dram[:].unsqueeze(0).to_broadcast([P, N])
            )

            # row scale from the local degrees (no rank dependence)
            r_sb = small.tile([P, S], F32)
            nc.vector.reciprocal(r_sb[:], dsb[:])
            nc.scalar.sqrt(r_sb[:], r_sb[:])

            # ---- pass 2: out = (A * r) * c ----
            # Loads on the sync ring, stores on the scalar ring: the two
            # HWDGE rings drain independently, so a load never queues
            # behind a store that waits on the multiply.
            for s in range(S):
                t = big.tile([P, N], F32, tag="stripe")
                nc.sync.dma_start(t[:], a[s * P : (s + 1) * P, :])
                nc.vector.scalar_tensor_tensor(
                    out=t[:],
                    in0=t[:],
                    scalar=r_sb[:, s : s + 1],
                    in1=cb[:],
                    op0=mybir.AluOpType.mult,
                    op1=mybir.AluOpType.mult,
                )
                nc.scalar.dma_start(out[s * P : (s + 1) * P, :], t[:])

    nc.compile()
    _CACHE["nc"] = nc
    return nc


def kernel(adjacency_matrix):
    A = np.ascontiguousarray(np.asarray(adjacency_matrix, dtype=np.float32))
    assert A.shape == (N, N)
    nc = build_nc()
    in_maps = [
        {"a_block": np.ascontiguousarray(A[k * R : (k + 1) * R])}
        for k in range(CORES)
    ]
    res = run_bass_kernel_spmd(nc, in_maps, list(range(CORES)))
    return np.concatenate(
        [res.results[k]["out_block"] for k in range(CORES)], axis=0
    )


# revision 9
# speedup vs baseline: 1.0988x; 1.0988x over previous
"""Laplacian normalization kernel for Trainium2 (8 NeuronCores, SPMD).

out = D^-1/2 A D^-1/2 where D = diag(row sums of A), A: [8192, 8192] fp32.

Sharding: rows are split across 8 cores (1024 rows each). Each core:
  pass 1: stream its row block, reduce row sums (local degrees)
  AllGather the [1024] local degree chunks -> full [8192] degree vector
  compute isq = 1/sqrt(deg); broadcast isq across partitions for the
  column scale; row scale comes from the local degrees directly
  pass 2: out = (A * r[:, None]) * c[None, :] as one fused DVE op per stripe
"""

import sys

sys.path.insert(0, "/opt/trn_rl_repo")

import numpy as np

import concourse.bacc as bacc
import concourse.tile as tile
from concourse import mybir
from concourse.bass_utils import run_bass_kernel_spmd

N = 8192          # full matrix dim
CORES = 8
R = N // CORES    # rows per core: 1024
P = 128           # partitions
S = R // P        # stripes per core: 8
F32 = mybir.dt.float32

_CACHE = {}


def build_nc():
    if "nc" in _CACHE:
        return _CACHE["nc"]
    nc = bacc.Bacc(
        "TRN2", target_bir_lowering=False, debug=False, num_devices=CORES
    )
    a = nc.dram_tensor("a_block", [R, N], F32, kind="ExternalInput").ap()
    out = nc.dram_tensor("out_block", [R, N], F32, kind="ExternalOutput").ap()

    with tile.TileContext(nc) as tc:
        with (
            tc.tile_pool(name="dram", bufs=1, space="DRAM") as dram,
            tc.tile_pool(name="big", bufs=5) as big,
            tc.tile_pool(name="cpool", bufs=1) as cpool,
            tc.tile_pool(name="small", bufs=1) as small,
        ):
            deg_loc = dram.tile([R], F32)
            deg_all = dram.tile([N], F32, addr_space="Shared")
            isq_dram = dram.tile([N], F32)

            # ---- pass 1: row sums of each 128-row stripe ----
            # Big loads go on the nc.sync HWDGE ring; the tiny degree
            # writes go via SWDGE (gpsimd) so the FIFO ring of big loads
            # never stalls behind a reduce-dependent small DMA.
            dsb = small.tile([P, S], F32)  # local degrees, stripe s in col s
            for s in range(S):
                t = big.tile([P, N], F32, tag="stripe")
                ld = nc.sync if s % 2 == 0 else nc.scalar
                ld.dma_start(t[:], a[s * P : (s + 1) * P, :])
                nc.vector.reduce_sum(
                    out=dsb[:, s : s + 1], in_=t[:], axis=mybir.AxisListType.X
                )
                nc.gpsimd.dma_start(
                    deg_loc[s * P : (s + 1) * P].unsqueeze(1), dsb[:, s : s + 1]
                )

            # ---- collective: full degree vector ----
            nc.gpsimd.collective_compute(
                "AllGather",
                mybir.AluOpType.bypass,
                replica_groups=[list(range(CORES))],
                ins=[deg_loc[:].opt()],
                outs=[deg_all[:].opt()],
            )

            # ---- isq = 1/sqrt(deg_all), staged back through DRAM ----
            # All on SWDGE: independent of the big-load ring, so these
            # latency-critical steps run as soon as the collective lands.
            t64 = small.tile([P, N // P], F32)
            nc.gpsimd.dma_start(t64[:], deg_all[:].rearrange("(p f) -> p f", p=P))
            nc.vector.reciprocal(t64[:], t64[:])
            nc.scalar.sqrt(t64[:], t64[:])
            nc.gpsimd.dma_start(
                isq_dram[:].rearrange("(p f) -> p f", p=P), t64[:]
            )

            # column scale broadcast across all 128 partitions
            cb = cpool.tile([P, N], F32)
            nc.gpsimd.dma_start(
                cb[:], isq_dram[:].unsqueeze(0).to_broadcast([P, N])
            )

            # row scale from the local degrees (no rank dependence)
            r_sb = small.tile([P, S], F32)
            nc.vector.reciprocal(r_sb[:], dsb[:])
            nc.scalar.sqrt(r_sb[:], r_sb[:])

            # ---- pass 2: out = (A * r) * c ----
            # Loads on the sync ring, stores on the scalar ring: the two
            # HWDGE rings drain independently, so a load never queues
            # behind a store that waits on the multiply.
            for s in range(S):
                t = big.tile([P, N], F32, tag="stripe")
                ld = nc.sync if s % 2 == 0 else nc.scalar
                st = nc.scalar if s % 2 == 0 else nc.sync
                ld.dma_start(t[:], a[s * P : (s + 1) * P, :])
                nc.vector.scalar_tensor_tensor(
                    out=t[:],
                    in0=t[:],
                    scalar=r_sb[:, s : s + 1],
                    in1=cb[:],
                    op0=mybir.AluOpType.mult,
                    op1=mybir.AluOpType.mult,
                )
                st.dma_start(out[s * P : (s + 1) * P, :], t[:])

    nc.compile()
    _CACHE["nc"] = nc
    return nc


def kernel(adjacency_matrix):
    A = np.ascontiguousarray(np.asarray(adjacency_matrix, dtype=np.float32))
    assert A.shape == (N, N)
    nc = build_nc()
    in_maps = [
        {"a_block": np.ascontiguousarray(A[k * R : (k + 1) * R])}
        for k in range(CORES)
    ]
    res = run_bass_kernel_spmd(nc, in_maps, list(range(CORES)))
    return np.concatenate(
        [res.results[k]["out_block"] for k in range(CORES)], axis=0
    )
